# revision 1
# baseline (speedup 1.0000x reference)
"""Trainium2 Bass kernel for per-pixel channel-mixing "attention".

Math per pixel (b,h,w), with q=x, k=y, v=z, all [B,C,H,W], C=64:
    kv[i,j] = v_i * k_j               (64x64 outer product)
    attn    = softmax over i          (column softmax)
    out_i   = sum_j attn[i,j] * q_j
            = sum_j exp(v_i*k_j) * (q_j / d_j),  d_j = sum_i exp(v_i*k_j)

Max-subtraction is skipped: |v_i*k_j| <= ~30 for randn inputs, far below
fp32 exp overflow (88), and exp/sum is mathematically identical to
jax.nn.softmax.

Layout strategy (per core, 8-way shard over the (b,h) axis):
  - Each core gets 32 contiguous (b,h) rows: inputs [64, 32, 128] fp32.
  - Tile = one h-row = 128 pixels. Pixels go on SBUF partitions (PE
    transposes in, PE transpose out), channels on the free dim, so all
    per-pixel quantities (k_j, d_j, w_j) are per-partition vectors.
  - P = v x k outer product in fp16 on DVE at 2x: the k-broadcast AP
    (outer stride-0) qualifies for the 2x mode directly; the v
    element-repeat does not, so V_exp is materialized by ACT (which
    accepts stride-0 input APs). E = exp(P) -> bf16, one big ACT instr.
  - d_j = sum_i E via a depth-4 bf16 pairwise tree on DVE (tensor_reduce
    is stuck at 1x; 2x tree adds + small fp32 strided reduce is faster).
  - w = q/d; out_i = sum_j E[:,i,:]*w_j: four bf16 2x multiplies by
    j-quarter, pairwise-summed by accumulate-DMAs (SWDGE CCE adds on idle
    DMA queues), then a small DVE tree + fp32 reduce.
  - Engine split per tile (measured): DVE ~9.5us, ACT ~8.5us, PE/DMA/
    GpSimd light. GpSimd shares SBUF ports with DVE (measured), so it
    cannot act as extra elementwise capacity.
"""

import sys

sys.path.insert(0, "/opt/trn_rl_repo")

from contextlib import ExitStack

import numpy as np

import concourse.bacc as bacc
import concourse.bass as bass
import concourse.tile as tile
from concourse import mybir
from concourse.bass_utils import run_bass_kernel_spmd
from concourse.masks import make_identity

B, C, H, W = 2, 64, 128, 128
N_CORES = 8
ROWS = B * H  # 256 (b,h) rows total
ROWS_PER_CORE = ROWS // N_CORES  # 32
NTILES = ROWS_PER_CORE  # one tile per h-row, 128 pixels each

FP32 = mybir.dt.float32
FP16 = mybir.dt.float16
BF16 = mybir.dt.bfloat16
EXP = mybir.ActivationFunctionType.Exp


def build_kernel():
    nc = bacc.Bacc(
        "TRN2",
        target_bir_lowering=False,
        debug=False,
        enable_asserts=False,
        num_devices=N_CORES,
    )
    xq = nc.dram_tensor("xq", [C, ROWS_PER_CORE, W], FP32, kind="ExternalInput").ap()
    yk = nc.dram_tensor("yk", [C, ROWS_PER_CORE, W], FP32, kind="ExternalInput").ap()
    zv = nc.dram_tensor("zv", [C, ROWS_PER_CORE, W], FP32, kind="ExternalInput").ap()
    out = nc.dram_tensor("out", [C, ROWS_PER_CORE, W], FP32, kind="ExternalOutput").ap()

    with tile.TileContext(nc) as tc, ExitStack() as ctx:
        singles = ctx.enter_context(tc.tile_pool(name="singles", bufs=1))
        big_in = ctx.enter_context(tc.tile_pool(name="big_in", bufs=4))
        psum = ctx.enter_context(tc.tile_pool(name="psum", bufs=2, space="PSUM"))
        psum_o = ctx.enter_context(tc.tile_pool(name="psum_o", bufs=2, space="PSUM"))
        tposed = ctx.enter_context(tc.tile_pool(name="tposed", bufs=4))
        bigs = ctx.enter_context(tc.tile_pool(name="bigs", bufs=2))
        bigs3 = ctx.enter_context(tc.tile_pool(name="bigs3", bufs=4))
        smalls = ctx.enter_context(tc.tile_pool(name="smalls", bufs=4))

        ident = singles.tile([128, 128], FP32)
        make_identity(nc, ident)

        for t in range(NTILES):
            # Per-tile input loads (a whole-shard preload would stall the
            # first tiles behind one 2MB DMA dependency).
            qn = big_in.tile([C, W], FP32, tag="qn")
            kn = big_in.tile([C, W], FP32, tag="kn")
            vn = big_in.tile([C, W], FP32, tag="vn")
            nc.sync.dma_start(out=qn, in_=xq[:, t, :])
            nc.sync.dma_start(out=kn, in_=yk[:, t, :])
            nc.sync.dma_start(out=vn, in_=zv[:, t, :])

            # Transpose [64ch, 128pix] -> [128pix, 64ch] on PE.
            q_ps = psum.tile([128, C], FP32, tag="qps")
            k_ps = psum.tile([128, C], FP32, tag="kps")
            v_ps = psum.tile([128, C], FP32, tag="vps")
            nc.tensor.transpose(q_ps, qn, ident[:C, :C])
            nc.tensor.transpose(k_ps, kn, ident[:C, :C])
            nc.tensor.transpose(v_ps, vn, ident[:C, :C])
            qT = tposed.tile([128, C], FP32, tag="qT")
            kT16 = tposed.tile([128, C], FP16, tag="kT16")
            vT16 = tposed.tile([128, C], FP16, tag="vT16")
            nc.scalar.copy(qT, q_ps)
            nc.scalar.copy(kT16, k_ps)
            nc.scalar.copy(vT16, v_ps)

            # P[pix, i, j] = v_i * k_j in fp16 at DVE 2x. The 2x mode only
            # needs every operand's LAST AP dim to be stride-1 with >=2
            # elements, so instead of materializing a full [128,C,C] v
            # broadcast, build a tiny pair-repeated v2 [128,C,2] (one small
            # ACT copy) and read both operands through 4D APs whose last dim
            # is a contiguous j-pair (measured: full 2x, 2203ns).
            v2 = tposed.tile([128, C, 2], FP16, tag="v2")
            nc.scalar.copy(v2, vT16[:, :, None].broadcast_to([128, C, 2]))
            P = bigs3.tile([128, C, C], FP16, tag="P")
            k_op = bass.AP(
                tensor=kT16.tensor,
                offset=kT16.offset,
                ap=[kT16.ap[0], [0, C], [2, C // 2], [1, 2]],
            )
            v_op = bass.AP(
                tensor=v2.tensor,
                offset=v2.offset,
                ap=[v2.ap[0], [2, C], [0, C // 2], [1, 2]],
            )
            nc.vector.tensor_mul(
                P.rearrange("p i (jh jp) -> p i jh jp", jp=2), k_op, v_op
            )
            # E = exp(P), bf16, one big ACT instruction.
            E = bigs3.tile([128, C, C], BF16, tag="E")
            nc.scalar.activation(out=E, in_=P, func=EXP)

            # d[pix, j] = sum_i E: depth-4 bf16 tree (2x DVE mode) + fp32
            # strided reduce over the remaining 4 i-slots. (The final reduce
            # reads stride-64, which measures ~1.8x slower per element —
            # hence the deep tree.)
            G1 = bigs3.tile([128, C // 4, C], BF16, tag="G1")
            G2 = bigs3.tile([128, C // 4, C], BF16, tag="G2")
            nc.vector.tensor_add(G1, E[:, : C // 4, :], E[:, C // 4 : C // 2, :])
            nc.vector.tensor_add(G2, E[:, C // 2 : 3 * C // 4, :], E[:, 3 * C // 4 :, :])
            nc.gpsimd.dma_start(out=G1, in_=G2, accum_op=mybir.AluOpType.add)
            nc.vector.tensor_add(
                G1[:, : C // 8, :], G1[:, : C // 8, :], G1[:, C // 8 : C // 4, :]
            )
            nc.vector.tensor_add(
                G1[:, : C // 16, :], G1[:, : C // 16, :], G1[:, C // 16 : C // 8, :]
            )
            d = smalls.tile([128, C], FP32, tag="d")
            nc.vector.tensor_reduce(
                out=d,
                in_=G1[:, : C // 16, :].transpose([0, 2, 1]),
                axis=mybir.AxisListType.X,
                op=mybir.AluOpType.add,
            )

            # w = q / d, written straight to bf16 by the multiply.
            r = smalls.tile([128, C], FP32, tag="r")
            nc.vector.reciprocal(r, d)
            w16 = smalls.tile([128, C], BF16, tag="w16")
            nc.vector.tensor_mul(w16, qT, r)

            # out_i = sum_j E[:, i, :] * w: bf16 2x multiplies (w broadcast
            # over i) emitted as four j-quarters, whose pairwise sums run as
            # accumulate-DMAs (SWDGE CCE adds, distinct src/dst tiles) on
            # otherwise-idle DMA queues — two j-halving levels off the DVE.
            Q = C // 4
            F1 = bigs3.tile([128, C, Q], BF16, tag="F1")
            F2 = bigs3.tile([128, C, Q], BF16, tag="F2")
            F3 = bigs3.tile([128, C, Q], BF16, tag="F3")
            F4 = bigs3.tile([128, C, Q], BF16, tag="F4")
            for fi, Fq in enumerate((F1, F2, F3, F4)):
                nc.vector.tensor_mul(
                    Fq,
                    E[:, :, fi * Q : (fi + 1) * Q],
                    w16[:, None, fi * Q : (fi + 1) * Q].broadcast_to([128, C, Q]),
                )
            nc.gpsimd.dma_start(out=F1, in_=F2, accum_op=mybir.AluOpType.add)
            nc.gpsimd.dma_start(out=F3, in_=F4, accum_op=mybir.AluOpType.add)
            nc.vector.tensor_add(F1, F1, F3)
            nc.vector.tensor_add(
                F1[:, :, : Q // 2], F1[:, :, : Q // 2], F1[:, :, Q // 2 :]
            )
            nc.vector.tensor_add(
                F1[:, :, : Q // 4], F1[:, :, : Q // 4], F1[:, :, Q // 4 : Q // 2]
            )
            oT = smalls.tile([128, C], FP32, tag="oT")
            nc.vector.tensor_reduce(
                out=oT,
                in_=F1[:, :, : Q // 4],
                axis=mybir.AxisListType.X,
                op=mybir.AluOpType.add,
            )

            # Transpose back [128pix, 64ch] -> [64ch, 128pix] and store.
            o_ps = psum_o.tile([C, 128], FP32, tag="ops")
            nc.tensor.transpose(o_ps, oT, ident)
            o_sb = tposed.tile([C, 128], FP32, tag="osb")
            nc.scalar.copy(o_sb, o_ps)
            nc.sync.dma_start(out=out[:, t, :], in_=o_sb)

    nc.compile()
    return nc


_NC_CACHE = None


def _get_nc():
    global _NC_CACHE
    if _NC_CACHE is None:
        _NC_CACHE = build_kernel()
    return _NC_CACHE


def _shard(a):
    # [B, C, H, W] -> per-core [C, 32, W], sharding flattened (b,h) rows.
    r = np.ascontiguousarray(np.transpose(np.asarray(a), (1, 0, 2, 3))).reshape(
        C, ROWS, W
    )
    return [
        np.ascontiguousarray(r[:, c * ROWS_PER_CORE : (c + 1) * ROWS_PER_CORE, :])
        for c in range(N_CORES)
    ]


def kernel(x, y, z):
    nc = _get_nc()
    xs, ys, zs = _shard(x), _shard(y), _shard(z)
    in_maps = [{"xq": xs[c], "yk": ys[c], "zv": zs[c]} for c in range(N_CORES)]
    res = run_bass_kernel_spmd(nc, in_maps, core_ids=list(range(N_CORES)))
    parts = [res.results[c]["out"] for c in range(N_CORES)]
    full = np.concatenate(parts, axis=1)  # [C, 256, W]
    return np.ascontiguousarray(
        np.transpose(full.reshape(C, B, H, W), (1, 0, 2, 3))
    ).astype(np.float32)

